# revision 1
# baseline (speedup 1.0000x reference)
"""Trainium2 Bass kernel for nn_AttentiveStylizationBlock (B=8,T=4096,E=1024,M=256,L=512).

Sharding: data-parallel over batch — core i computes batch element i entirely
(weights replicated, no collectives).

Math per batch element (algebraically refactored from the reference):
    k   = latent @ Wk + bk                      [M, E]
    v   = latent @ Wv + bv                      [M, E]
    kq  = Wq @ k^T                              [E, M]   (folds the q-projection:
          w = (emb Wq + bq) k^T = emb . kq + bq . k^T)
    c   = (bq . k^T) / sqrt(E)                  [M]
    ew[m,t] = exp(kq[:,m] . emb[t,:] / sqrt(E) + c[m])
    S[m]    = sum_t ew[m,t]                     (softmax over frames T, dim=1)
    vn  = v / S[:, None]
    pred[t] = sum_m ew[m,t] * vn[m]             [T, E]
    out = LN(pred + emb) * gamma + beta

w values are ~N(0,1) (|w| < 6 measured), so exp without max-subtraction is safe.
"""

import os
import sys

sys.path.insert(0, "/opt/trn_rl_repo")

import numpy as np

B, T, E, M, L = 8, 4096, 1024, 256, 512
P = 128
EPS = 1e-6
ES = E // P        # 8  e-subtiles
LS = L // P        # 4  l-subtiles
MB = M // P        # 2  m-blocks
TT = 512           # t-tile (free dim of the big matmuls)
NT = T // TT       # 8  t-tiles
TS = TT // P       # 4  t-subblocks per t-tile
EH = E // 512      # 2  e-halves (psum free-dim limit for fp32)
SCALE = 1.0 / float(np.sqrt(E))

# Matmul compute mode: "f32" (exact, 4 cyc/row) or "f32r" (fp32 fast mode, 1 cyc/row)
MM_MODE = os.environ.get("KERNEL_MM_MODE", "f32r")
# Repeat the whole body inside one NEFF (for differential on-device timing)
REPS = int(os.environ.get("KERNEL_REPS", "1"))

_cache = {}
LAST_RUN = {}


def _bcast_ap(ap, p):
    """[free...] DRAM AP -> [p, free...] partition-broadcast AP."""
    import concourse.bass as bass

    return bass.AP(tensor=ap.tensor, offset=ap.offset, ap=[[0, p], *ap.ap])


def _build(reps=None):
    if reps is None:
        reps = REPS
    import concourse.bacc as bacc
    import concourse.mybir as mybir
    import concourse.tile as tile
    from concourse.masks import make_identity

    f32 = mybir.dt.float32
    mmdt = {"f32r": mybir.dt.float32r,
            "bf16": mybir.dt.bfloat16}.get(MM_MODE, f32)
    BF16 = MM_MODE == "bf16"
    AF = mybir.ActivationFunctionType
    nc = bacc.Bacc(None, target_bir_lowering=False)

    emb = nc.dram_tensor("emb", (T, E), f32, kind="ExternalInput")
    latent = nc.dram_tensor("latent", (M, L), f32, kind="ExternalInput")
    Wq = nc.dram_tensor("Wq", (E, E), f32, kind="ExternalInput")
    bq = nc.dram_tensor("bq", (E,), f32, kind="ExternalInput")
    Wk = nc.dram_tensor("Wk", (L, E), f32, kind="ExternalInput")
    bk = nc.dram_tensor("bk", (E,), f32, kind="ExternalInput")
    Wv = nc.dram_tensor("Wv", (L, E), f32, kind="ExternalInput")
    bv = nc.dram_tensor("bv", (E,), f32, kind="ExternalInput")
    gamma = nc.dram_tensor("gamma", (E,), f32, kind="ExternalInput")
    beta = nc.dram_tensor("beta", (E,), f32, kind="ExternalInput")
    out = nc.dram_tensor("out", (T, E), f32, kind="ExternalOutput")

    nb = 2 if BF16 else 3
    with tile.TileContext(nc) as tc, \
         tc.tile_pool(name="const", bufs=1) as const, \
         tc.tile_pool(name="persist", bufs=1) as persist, \
         tc.tile_pool(name="wload", bufs=nb) as wload, \
         tc.tile_pool(name="trans", bufs=2) as trans, \
         tc.tile_pool(name="stream", bufs=2) as stream, \
         tc.tile_pool(name="embtp", bufs=2) as embtp, \
         tc.tile_pool(name="small", bufs=2 if BF16 else 4) as small, \
         tc.tile_pool(name="psum_tr", bufs=3, space="PSUM") as psum_tr, \
         tc.tile_pool(name="psum_mm", bufs=4, space="PSUM") as psum_mm:

        # ---- constants ----
        ident = const.tile([P, P], f32)
        make_identity(nc, ident)
        if BF16:
            ident_t = const.tile([P, P], mmdt, tag="identbf")
            make_identity(nc, ident_t)
        else:
            ident_t = ident
        eps_t = const.tile([P, 1], f32)
        nc.vector.memset(eps_t, EPS)
        gamma_bc = const.tile([P, E], f32)
        nc.gpsimd.dma_start(out=gamma_bc, in_=_bcast_ap(gamma[:], P))
        beta_bc = const.tile([P, E], f32)
        nc.gpsimd.dma_start(out=beta_bc, in_=_bcast_ap(beta[:], P))
        bv_bc = const.tile([P, E], f32)
        nc.gpsimd.dma_start(out=bv_bc, in_=_bcast_ap(bv[:], P))
        bq_pp = const.tile([P, ES], f32)
        nc.sync.dma_start(bq_pp, bq[:].rearrange("(o p) -> p o", p=P))
        bk_pp = const.tile([P, ES], f32)
        nc.sync.dma_start(bk_pp, bk[:].rearrange("(o p) -> p o", p=P))

        for _rep in range(reps):
            # ---- latent^T  [l, m] ----
            lat_nat = persist.tile([P, MB, L], f32,
                                   tag="scratch2k" if BF16 else "latnat")
            nc.sync.dma_start(lat_nat, latent[:, :].rearrange("(mb p) l -> p mb l", p=P))
            latT = persist.tile([P, LS, M], mmdt)
            for mb in range(MB):
                pst = psum_tr.tile([P, TT], f32, tag="tr")
                for ls in range(LS):
                    nc.tensor.transpose(pst[:, ls * P:(ls + 1) * P],
                                        lat_nat[:, mb, ls * P:(ls + 1) * P], ident)
                nc.vector.tensor_copy(
                    out=latT[:, :, mb * P:(mb + 1) * P],
                    in_=pst.rearrange("p (ls m) -> p ls m", ls=LS))

            # ---- k^T  [e, m] = Wk^T latT + bk ----
            wk_sb = persist.tile([P, LS, E], mmdt, tag="wksb")
            nc.gpsimd.dma_start(wk_sb, Wk[:, :].rearrange("(lo p) e -> p lo e", p=P))
            k_f32 = persist.tile([P, ES, M], f32, tag="scratch2k")
            k_em = persist.tile([P, ES, M], mmdt)
            for es in range(ES):
                ps = psum_mm.tile([P, 512], f32, tag="mm")
                for ls in range(LS):
                    nc.tensor.matmul(ps[:, :M], wk_sb[:, ls, es * P:(es + 1) * P],
                                     latT[:, ls, :],
                                     start=(ls == 0), stop=(ls == LS - 1))
                nc.scalar.activation(k_f32[:, es, :], ps[:, :M], AF.Identity,
                                     bias=bk_pp[:, es:es + 1])
                nc.vector.tensor_copy(out=k_em[:, es, :], in_=k_f32[:, es, :])

            # ---- kq [e_in, m] = Wq @ k^T  (needs Wq^T tiles via PE transpose) ----
            kq = persist.tile([P, ES, M], mmdt)
            for eb in range(ES):
                wq_row = wload.tile([P, E], f32, tag="wqrow")
                nc.sync.dma_start(wq_row, Wq[eb * P:(eb + 1) * P, :])
                wqT_row = trans.tile([P, ES, P], mmdt, tag="wqTrow")
                for half in range(2):
                    pst = psum_tr.tile([P, TT], f32, tag="tr")
                    for j in range(4):
                        fs = half * 4 + j
                        nc.tensor.transpose(pst[:, j * P:(j + 1) * P],
                                            wq_row[:, fs * P:(fs + 1) * P], ident)
                    nc.vector.tensor_copy(
                        out=wqT_row[:, half * 4:(half + 1) * 4, :],
                        in_=pst.rearrange("p (j m) -> p j m", j=4))
                ps = psum_mm.tile([P, 512], f32, tag="mm")
                for fs in range(ES):
                    nc.tensor.matmul(ps[:, :M], wqT_row[:, fs, :], k_em[:, fs, :],
                                     start=(fs == 0), stop=(fs == ES - 1))
                nc.scalar.copy(kq[:, eb, :], ps[:, :M])

            # ---- c [m] = (bq . k^T) * SCALE  (per-partition layout, per m-block) ----
            c_pp = const.tile([P, MB], f32)
            for mb in range(MB):
                ps = psum_mm.tile([P, 512], f32, tag="mm")
                for fs in range(ES):
                    nc.tensor.matmul(ps[:, :1], k_f32[:, fs, mb * P:(mb + 1) * P],
                                     bq_pp[:, fs:fs + 1],
                                     start=(fs == 0), stop=(fs == ES - 1))
                nc.scalar.mul(c_pp[:, mb:mb + 1], ps[:, :1], SCALE)

            # ---- pass 1 over T: exp_wT [m, t] and row sums ----
            exp_wT = persist.tile([P, MB, T], mmdt)
            s_part = persist.tile([P, MB, NT], f32)
            if BF16:
                # bf16 stash of emb: avoids re-loading emb from HBM in pass 2
                emb_bf = persist.tile([P, T // P, E], mmdt, tag="embbf")
            for it in range(NT):
                emb_nat = stream.tile([P, TS, E], f32, tag="embL")
                nc.sync.dma_start(
                    emb_nat,
                    emb[it * TT:(it + 1) * TT, :].rearrange("(ts p) e -> p ts e", p=P))
                if BF16:
                    nc.gpsimd.tensor_copy(
                        out=emb_bf[:, it * TS:(it + 1) * TS, :], in_=emb_nat)
                    tr_src = emb_bf[:, it * TS:(it + 1) * TS, :]
                else:
                    tr_src = emb_nat
                embT = embtp.tile([P, ES, TT], mmdt, tag="bigshare")
                for es in range(ES):
                    pst = psum_tr.tile([P, TT], mmdt if BF16 else f32, tag="tr")
                    for ts in range(TS):
                        nc.tensor.transpose(pst[:, ts * P:(ts + 1) * P],
                                            tr_src[:, ts, es * P:(es + 1) * P], ident_t)
                    nc.vector.tensor_copy(out=embT[:, es, :], in_=pst)
                for mb in range(MB):
                    psw = psum_mm.tile([P, 512], f32, tag="mm")
                    for es in range(ES):
                        nc.tensor.matmul(psw[:, :TT], kq[:, es, mb * P:(mb + 1) * P],
                                         embT[:, es, :],
                                         start=(es == 0), stop=(es == ES - 1))
                    nc.scalar.activation(exp_wT[:, mb, it * TT:(it + 1) * TT], psw[:, :TT],
                                         AF.Exp, bias=c_pp[:, mb:mb + 1], scale=SCALE,
                                         accum_out=s_part[:, mb, it:it + 1])

            # ---- softmax denominators and normalized v ----
            s_tot = small.tile([P, MB, 1], f32, tag="stot")
            nc.vector.reduce_sum(s_tot, s_part, axis=mybir.AxisListType.X)
            inv_s = small.tile([P, MB, 1], f32, tag="invs")
            nc.vector.reciprocal(inv_s, s_tot)

            # v [m, e] = latT^T Wv + bv, then scale rows by 1/S
            v_norm = persist.tile([P, MB, E], mmdt)
            v_tmp = persist.tile([P, MB, E], f32, tag="scratch2k")
            wv_sb = persist.tile([P, LS, E], mmdt, tag="wvsb")
            nc.gpsimd.dma_start(wv_sb, Wv[:, :].rearrange("(lo p) e -> p lo e", p=P))
            for eh in range(EH):
                for mb in range(MB):
                    ps = psum_mm.tile([P, 512], f32, tag="mm")
                    for ls in range(LS):
                        nc.tensor.matmul(ps, latT[:, ls, mb * P:(mb + 1) * P],
                                         wv_sb[:, ls, eh * 512:(eh + 1) * 512],
                                         start=(ls == 0), stop=(ls == LS - 1))
                    nc.vector.tensor_add(v_tmp[:, mb, eh * 512:(eh + 1) * 512], ps,
                                         bv_bc[:, eh * 512:(eh + 1) * 512])
            for mb in range(MB):
                nc.vector.tensor_scalar_mul(v_norm[:, mb, :], v_tmp[:, mb, :],
                                            inv_s[:, mb, :])

            # ---- pass 2 over T: pred + residual + LayerNorm ----
            for it in range(NT):
                if BF16:
                    emb2 = emb_bf[:, it * TS:(it + 1) * TS, :]
                else:
                    emb2 = stream.tile([P, TS, E], f32, tag="embL")
                    nc.sync.dma_start(
                        emb2,
                        emb[it * TT:(it + 1) * TT, :].rearrange("(ts p) e -> p ts e", p=P))
                xout = embtp.tile([P, TS, E], f32, tag="bigshare")
                for ts in range(TS):
                    t0 = it * TT + ts * P
                    racc = small.tile([P, EH], f32, tag="racc")
                    for eh in range(EH):
                        psp = psum_mm.tile([P, 512], f32, tag="mm")
                        for mb in range(MB):
                            nc.tensor.matmul(psp, exp_wT[:, mb, t0:t0 + P],
                                             v_norm[:, mb, eh * 512:(eh + 1) * 512],
                                             start=(mb == 0), stop=(mb == MB - 1))
                        # x = pred + emb, with free row-sum accumulated for the mean
                        nc.vector.scalar_tensor_tensor(
                            out=xout[:, ts, eh * 512:(eh + 1) * 512],
                            in0=psp, scalar=1.0,
                            in1=emb2[:, ts, eh * 512:(eh + 1) * 512],
                            op0=mybir.AluOpType.mult, op1=mybir.AluOpType.add,
                            accum_out=racc[:, eh:eh + 1])
                    # LayerNorm over E (free axis): mean from racc, sum(x^2) via
                    # a Square activation with accumulate (scalar engine)
                    sqs = persist.tile([P, E], f32, tag="scratch2k")
                    ssq = small.tile([P, 1], f32, tag="ssq")
                    nc.scalar.activation(sqs, xout[:, ts, :], AF.Square,
                                         accum_out=ssq)
                    mu = small.tile([P, 1], f32, tag="mu")
                    nc.vector.scalar_tensor_tensor(
                        out=mu, in0=racc[:, 0:1], scalar=1.0, in1=racc[:, 1:2],
                        op0=mybir.AluOpType.mult, op1=mybir.AluOpType.add)
                    nc.vector.tensor_scalar_mul(mu, mu, 1.0 / E)
                    musq = small.tile([P, 1], f32, tag="musq")
                    nc.vector.tensor_mul(musq, mu, mu)
                    var = small.tile([P, 1], f32, tag="var")
                    nc.vector.scalar_tensor_tensor(
                        out=var, in0=ssq, scalar=1.0 / E, in1=musq,
                        op0=mybir.AluOpType.mult, op1=mybir.AluOpType.subtract)
                    rstd = small.tile([P, 1], f32, tag="rstd")
                    nc.scalar.activation(rstd, var, AF.Sqrt, bias=eps_t, scale=1.0)
                    nc.vector.reciprocal(rstd, rstd)
                    # xhat = x*rstd - mu*rstd on the scalar engine
                    nmr = small.tile([P, 1], f32, tag="nmr")
                    nc.vector.tensor_scalar(nmr, in0=mu, scalar1=rstd,
                                            scalar2=-1.0,
                                            op0=mybir.AluOpType.mult,
                                            op1=mybir.AluOpType.mult)
                    nc.scalar.activation(xout[:, ts, :], xout[:, ts, :], AF.Identity,
                                         bias=nmr, scale=rstd)
                    nc.vector.tensor_mul(xout[:, ts, :], xout[:, ts, :], gamma_bc)
                    nc.gpsimd.tensor_add(xout[:, ts, :], xout[:, ts, :], beta_bc)
                nc.sync.dma_start(
                    out[it * TT:(it + 1) * TT, :].rearrange("(ts p) e -> p ts e", p=P),
                    xout)

    nc.compile()
    return nc


def kernel(emb, latent, Wq, bq, Wk, bk, Wv, bv, gamma, beta):
    from concourse.bass_utils import run_bass_kernel_spmd

    if "nc" not in _cache:
        _cache["nc"] = _build()
    nc = _cache["nc"]

    emb = np.ascontiguousarray(emb, dtype=np.float32)
    latent = np.ascontiguousarray(latent, dtype=np.float32)
    shared = {
        "Wq": np.ascontiguousarray(Wq, dtype=np.float32),
        "bq": np.ascontiguousarray(bq, dtype=np.float32),
        "Wk": np.ascontiguousarray(Wk, dtype=np.float32),
        "bk": np.ascontiguousarray(bk, dtype=np.float32),
        "Wv": np.ascontiguousarray(Wv, dtype=np.float32),
        "bv": np.ascontiguousarray(bv, dtype=np.float32),
        "gamma": np.ascontiguousarray(gamma, dtype=np.float32),
        "beta": np.ascontiguousarray(beta, dtype=np.float32),
    }
    in_maps = [
        {"emb": emb[b], "latent": latent[b], **shared} for b in range(B)
    ]
    trace = bool(int(os.environ.get("KERNEL_TRACE", "0")))
    res = run_bass_kernel_spmd(nc, in_maps, list(range(B)), trace=trace)
    LAST_RUN["exec_time_ns"] = res.exec_time_ns
    LAST_RUN["mean_exec_time_ns"] = res.mean_exec_time_ns
    LAST_RUN["profile_json"] = res.profile_json
    return np.stack([res.results[b]["out"] for b in range(B)], axis=0)



# revision 15
# speedup vs baseline: 1.1635x; 1.1635x over previous
"""Trainium2 Bass kernel for nn_AttentiveStylizationBlock (B=8,T=4096,E=1024,M=256,L=512).

Sharding: data-parallel over batch - core i computes batch element i entirely
(weights replicated, no collectives).

Math per batch element (algebraically refactored from the reference):
    k   = latent @ Wk + bk                      [M, E]
    v   = latent @ Wv + bv                      [M, E]
    kq  = Wq @ k^T                              [E, M]   (folds the q-projection:
          w = (emb Wq + bq) k^T = emb . kq + bq . k^T)
    c   = (bq . k^T) / sqrt(E)                  [M]
    ew[m,t] = exp(kq[:,m] . emb[t,:] / sqrt(E) + c[m])
    S[m]    = sum_t ew[m,t]                     (softmax over frames T, dim=1)
    vn  = v / S[:, None]
    pred[t] = sum_m ew[m,t] * vn[m]             [T, E]
    out = LN(pred + emb) * gamma + beta

Implementation notes (v2):
  - All matmul operands are bf16 (psum accumulation stays f32); emb is
    cast to bf16 in-flight by the gpsimd (SWDGE) DMA and stashed in SBUF
    for the whole kernel, so emb is read from HBM exactly once.
  - The residual add (pred + emb) is done on the PE via an extra
    identity-matmul accumulation into the same psum.
  - mean(x) is assembled from matmuls: sum_e pred = ew . rowsum(vn), and
    sum_e emb is reduced once per tile in pass 1 on gpsimd, so pass 2
    never does a separate sum pass over x.
  - sum(x^2) is one DVE tensor_tensor_reduce straight from psum; the
    normalize+evacuate is a single scalar-engine activation
    (out = x*rstd - mu*rstd) reading psum, writing SBUF f32.
  - gamma/beta application is skipped when gamma==1, beta==0 (values
    checked at run time; a general variant is compiled on demand).
"""

import os
import sys

sys.path.insert(0, "/opt/trn_rl_repo")

import numpy as np

B, T, E, M, L = 8, 4096, 1024, 256, 512
P = 128
EPS = 1e-6
ES = E // P        # 8  e-subtiles
LS = L // P        # 4  l-subtiles
MB = M // P        # 2  m-blocks
TT = 512           # t-tile (free dim of the big matmuls)
NT = T // TT       # 8  t-tiles
TS = TT // P       # 4  t-subblocks per t-tile
NTS = T // P       # 32 t-subblocks total
EH = E // 512      # 2  e-halves (psum free-dim limit for fp32)
SCALE = 1.0 / float(np.sqrt(E))

REPS = int(os.environ.get("KERNEL_REPS", "1"))

_cache = {}
LAST_RUN = {}


def _bcast_ap(ap, p):
    """[free...] DRAM AP -> [p, free...] partition-broadcast AP."""
    import concourse.bass as bass

    return bass.AP(tensor=ap.tensor, offset=ap.offset, ap=[[0, p], *ap.ap])


def _build(apply_affine, reps=None):
    if reps is None:
        reps = REPS
    import concourse.bacc as bacc
    import concourse.mybir as mybir
    import concourse.tile as tile
    from concourse.masks import make_identity

    f32 = mybir.dt.float32
    bf16 = mybir.dt.bfloat16
    AF = mybir.ActivationFunctionType
    OP = mybir.AluOpType
    nc = bacc.Bacc(None, target_bir_lowering=False)

    emb = nc.dram_tensor("emb", (T, E), f32, kind="ExternalInput")
    latent = nc.dram_tensor("latent", (M, L), f32, kind="ExternalInput")
    Wq = nc.dram_tensor("Wq", (E, E), f32, kind="ExternalInput")
    bq = nc.dram_tensor("bq", (E,), f32, kind="ExternalInput")
    Wk = nc.dram_tensor("Wk", (L, E), f32, kind="ExternalInput")
    bk = nc.dram_tensor("bk", (E,), f32, kind="ExternalInput")
    Wv = nc.dram_tensor("Wv", (L, E), f32, kind="ExternalInput")
    bv = nc.dram_tensor("bv", (E,), f32, kind="ExternalInput")
    gamma = nc.dram_tensor("gamma", (E,), f32, kind="ExternalInput")
    beta = nc.dram_tensor("beta", (E,), f32, kind="ExternalInput")
    out = nc.dram_tensor("out", (T, E), f32, kind="ExternalOutput")

    with tile.TileContext(nc) as tc, \
         tc.tile_pool(name="const", bufs=1) as const, \
         tc.tile_pool(name="persist", bufs=1) as persist, \
         tc.tile_pool(name="wload", bufs=2) as wload, \
         tc.tile_pool(name="embtp", bufs=2) as embtp, \
         tc.tile_pool(name="xout", bufs=3) as xoutp, \
         tc.tile_pool(name="small", bufs=4) as small, \
         tc.tile_pool(name="psum_tr", bufs=2, space="PSUM") as psum_tr, \
         tc.tile_pool(name="psum_mm", bufs=4, space="PSUM") as psum_mm, \
         tc.tile_pool(name="psum_sm", bufs=2, space="PSUM") as psum_sm:

        # ---- constants ----
        ident = const.tile([P, P], bf16)
        make_identity(nc, ident)
        bq_bf = const.tile([P, ES], bf16)
        nc.gpsimd.dma_start(out=bq_bf, in_=bq[:].rearrange("(o p) -> p o", p=P))
        bk_pp = const.tile([P, ES], f32)
        nc.sync.dma_start(bk_pp, bk[:].rearrange("(o p) -> p o", p=P))
        bv_bc = const.tile([P, E], f32)
        nc.gpsimd.dma_start(out=bv_bc, in_=_bcast_ap(bv[:], P))
        if apply_affine:
            gamma_bc = const.tile([P, E], f32)
            nc.gpsimd.dma_start(out=gamma_bc, in_=_bcast_ap(gamma[:], P))
            beta_bc = const.tile([P, E], f32)
            nc.gpsimd.dma_start(out=beta_bc, in_=_bcast_ap(beta[:], P))

        for _rep in range(reps):
            # ---- bf16 weight loads (SWDGE cast-DMAs, in priority order) ----
            lat_bf = persist.tile([P, MB, L], bf16, tag="latbf")
            nc.gpsimd.dma_start(
                out=lat_bf, in_=latent[:, :].rearrange("(mb p) l -> p mb l", p=P))
            wk_bf = persist.tile([P, LS, E], bf16, tag="wkbf")
            nc.gpsimd.dma_start(
                out=wk_bf, in_=Wk[:, :].rearrange("(lo p) e -> p lo e", p=P))

            # emb: cast-DMA the whole [T, E] into a bf16 SBUF stash, one
            # tile per t-block of 512; interleave the remaining weight
            # loads into the same SWDGE queue so they arrive on time.
            emb_bf = persist.tile([P, NTS, E], bf16, tag="embbf")
            wq_bf = persist.tile([P, ES, E], bf16, tag="wqbf")
            wv_bf = persist.tile([P, LS, E], bf16, tag="wvbf")

            def emb_tile_dma(it):
                nc.gpsimd.dma_start(
                    out=emb_bf[:, it * TS:(it + 1) * TS, :],
                    in_=emb[it * TT:(it + 1) * TT, :].rearrange(
                        "(ts p) e -> p ts e", p=P))

            emb_tile_dma(0)
            nc.gpsimd.dma_start(
                out=wq_bf, in_=Wq[:, :].rearrange("(eb p) e -> p eb e", p=P))
            emb_tile_dma(1)
            nc.gpsimd.dma_start(
                out=wv_bf, in_=Wv[:, :].rearrange("(lo p) e -> p lo e", p=P))
            for it in range(2, NT):
                emb_tile_dma(it)

            # ---- latent^T  [l, m] ----
            latT = persist.tile([P, LS, M], bf16, tag="latT")
            for mb in range(MB):
                pst = psum_tr.tile([P, TT], bf16, tag="tr")
                for ls in range(LS):
                    nc.tensor.transpose(pst[:, ls * P:(ls + 1) * P],
                                        lat_bf[:, mb, ls * P:(ls + 1) * P], ident)
                nc.vector.tensor_copy(
                    out=latT[:, :, mb * P:(mb + 1) * P],
                    in_=pst.rearrange("p (ls m) -> p ls m", ls=LS))

            # ---- k^T [e, m] = Wk^T latT (+bk), bf16 ----
            k_bf = persist.tile([P, ES, M], bf16, tag="kbf")
            for es in range(ES):
                ps = psum_mm.tile([P, 512], f32, tag="mm")
                for ls in range(LS):
                    nc.tensor.matmul(ps[:, :M], wk_bf[:, ls, es * P:(es + 1) * P],
                                     latT[:, ls, :],
                                     start=(ls == 0), stop=(ls == LS - 1))
                nc.scalar.activation(k_bf[:, es, :], ps[:, :M], AF.Identity,
                                     bias=bk_pp[:, es:es + 1])

            # ---- c [m] = (bq . k^T) * SCALE ----
            c_pp = const.tile([P, MB], f32)
            for mb in range(MB):
                ps = psum_sm.tile([P, 1], f32, tag="smm")
                for es in range(ES):
                    nc.tensor.matmul(ps, k_bf[:, es, mb * P:(mb + 1) * P],
                                     bq_bf[:, es:es + 1],
                                     start=(es == 0), stop=(es == ES - 1))
                nc.scalar.mul(c_pp[:, mb:mb + 1], ps, SCALE)

            # ---- kq [e_in, m] = Wq @ k^T (WqT tiles via PE transpose) ----
            kq = persist.tile([P, ES, M], bf16, tag="kq")
            for eb in range(ES):
                wqT_row = wload.tile([P, ES, P], bf16, tag="wqTrow")
                for half in range(2):
                    pst = psum_tr.tile([P, TT], bf16, tag="tr")
                    for j in range(4):
                        fs = half * 4 + j
                        nc.tensor.transpose(pst[:, j * P:(j + 1) * P],
                                            wq_bf[:, eb, fs * P:(fs + 1) * P], ident)
                    nc.vector.tensor_copy(
                        out=wqT_row[:, half * 4:(half + 1) * 4, :],
                        in_=pst.rearrange("p (j m) -> p j m", j=4))
                ps = psum_mm.tile([P, 512], f32, tag="mm")
                for fs in range(ES):
                    nc.tensor.matmul(ps[:, :M], wqT_row[:, fs, :], k_bf[:, fs, :],
                                     start=(fs == 0), stop=(fs == ES - 1))
                nc.scalar.copy(kq[:, eb, :], ps[:, :M])

            # ---- v [m, e] = latT^T Wv + bv (unnormalized), row sums ----
            v_bf = persist.tile([P, MB, E], bf16, tag="vbf")
            vras = small.tile([P, MB, EH], f32, tag="vras")
            for mb in range(MB):
                for eh in range(EH):
                    ps = psum_mm.tile([P, 512], f32, tag="mm")
                    for ls in range(LS):
                        nc.tensor.matmul(ps, latT[:, ls, mb * P:(mb + 1) * P],
                                         wv_bf[:, ls, eh * 512:(eh + 1) * 512],
                                         start=(ls == 0), stop=(ls == LS - 1))
                    nc.vector.scalar_tensor_tensor(
                        out=v_bf[:, mb, eh * 512:(eh + 1) * 512],
                        in0=ps, scalar=1.0,
                        in1=bv_bc[:, eh * 512:(eh + 1) * 512],
                        op0=OP.mult, op1=OP.add,
                        accum_out=vras[:, mb, eh:eh + 1])

            # ---- pass 1 over T: exp_wT [m, t], row sums, emb row sums ----
            exp_wT = persist.tile([P, MB, T], bf16, tag="expw")
            s_part = persist.tile([P, MB, NT], f32, tag="spart")
            esum = persist.tile([P, NTS, 1], f32, tag="esum")
            for it in range(NT):
                embT = embtp.tile([P, ES, TT], bf16, tag="embT")
                for es in range(ES):
                    pst = psum_tr.tile([P, TT], bf16, tag="tr")
                    for ts in range(TS):
                        nc.tensor.transpose(
                            pst[:, ts * P:(ts + 1) * P],
                            emb_bf[:, it * TS + ts, es * P:(es + 1) * P], ident)
                    if es % 2 == 0:
                        nc.vector.tensor_copy(out=embT[:, es, :], in_=pst)
                    else:
                        nc.scalar.copy(embT[:, es, :], pst)
                # sum_e emb for the mean
                nc.vector.reduce_sum(
                    esum[:, it * TS:(it + 1) * TS, :],
                    emb_bf[:, it * TS:(it + 1) * TS, :],
                    axis=mybir.AxisListType.X)
                for mb in range(MB):
                    psw = psum_mm.tile([P, 512], f32, tag="mm")
                    for es in range(ES):
                        nc.tensor.matmul(psw, kq[:, es, mb * P:(mb + 1) * P],
                                         embT[:, es, :],
                                         start=(es == 0), stop=(es == ES - 1))
                    nc.scalar.activation(exp_wT[:, mb, it * TT:(it + 1) * TT], psw,
                                         AF.Exp, bias=c_pp[:, mb:mb + 1], scale=SCALE,
                                         accum_out=s_part[:, mb, it:it + 1])

            # ---- softmax denominators; normalized v and its row sums ----
            s_tot = small.tile([P, MB, 1], f32, tag="stot")
            nc.vector.reduce_sum(s_tot, s_part, axis=mybir.AxisListType.X)
            inv_s = small.tile([P, MB, 1], f32, tag="invs")
            nc.vector.reciprocal(inv_s, s_tot)
            v_norm = persist.tile([P, MB, E], bf16, tag="vnorm")
            vsum_n = small.tile([P, MB, 1], bf16, tag="vsumn")
            vsum_f = small.tile([P, MB, 1], f32, tag="vsumf")
            for mb in range(MB):
                nc.vector.tensor_scalar_mul(v_norm[:, mb, :], v_bf[:, mb, :],
                                            inv_s[:, mb, :])
                nc.vector.scalar_tensor_tensor(
                    out=vsum_f[:, mb, :], in0=vras[:, mb, 0:1],
                    scalar=1.0, in1=vras[:, mb, 1:2], op0=OP.mult, op1=OP.add)
                nc.vector.tensor_scalar_mul(vsum_n[:, mb, :], vsum_f[:, mb, :],
                                            inv_s[:, mb, :])

            # ---- pass 2 over T: pred + residual + LayerNorm, fused ----
            eps_c = const.tile([P, 1], f32)
            nc.vector.memset(eps_c, EPS)
            esum_s = persist.tile([P, NTS, 1], f32, tag="esums")
            nc.vector.tensor_scalar_mul(esum_s, esum, 1.0 / E)
            for tsb in range(NTS):
                t0 = tsb * P
                # pred psum per e-half; the identity matmul folds in +emb
                psp = []
                for eh in range(EH):
                    pp = psum_mm.tile([P, 512], f32, tag="mm")
                    for mb in range(MB):
                        nc.tensor.matmul(pp, exp_wT[:, mb, t0:t0 + P],
                                         v_norm[:, mb, eh * 512:(eh + 1) * 512],
                                         start=(mb == 0), stop=False)
                    nc.tensor.matmul(pp, ident,
                                     emb_bf[:, tsb, eh * 512:(eh + 1) * 512],
                                     start=False, stop=True)
                    psp.append(pp)
                # sum_e pred[t] via matmul with vn row sums
                psmu = psum_sm.tile([P, 1], f32, tag="smm")
                for mb in range(MB):
                    nc.tensor.matmul(psmu, exp_wT[:, mb, t0:t0 + P],
                                     vsum_n[:, mb, :],
                                     start=(mb == 0), stop=(mb == MB - 1))
                # mu = (sum_e pred)/E + (sum_e emb)/E   (tiny ops)
                mu = small.tile([P, 1], f32, tag="mu")
                nc.vector.scalar_tensor_tensor(
                    out=mu, in0=psmu, scalar=1.0 / E, in1=esum_s[:, tsb, :],
                    op0=OP.mult, op1=OP.add)
                # sum_e x^2: Square on the scalar engine straight from psum
                # (a DVE op may read at most one PSUM operand)
                sqs = small.tile([P, 512], bf16, tag="sqscratch")
                ssq = small.tile([P, 2], f32, tag="ssq")
                for eh in range(EH):
                    nc.scalar.activation(sqs, psp[eh], AF.Square,
                                         accum_out=ssq[:, eh:eh + 1])
                # var = (ssq0+ssq1)/E - mu^2 ; rstd = 1/sqrt(var+eps)
                mu2 = small.tile([P, 1], f32, tag="mu2")
                nc.gpsimd.tensor_mul(mu2, mu, mu)
                s2 = small.tile([P, 1], f32, tag="s2")
                nc.gpsimd.tensor_add(s2, ssq[:, 0:1], ssq[:, 1:2])
                veps = small.tile([P, 1], f32, tag="veps")
                nc.vector.scalar_tensor_tensor(
                    out=veps, in0=s2, scalar=1.0 / E, in1=mu2,
                    op0=OP.mult, op1=OP.subtract)
                sstd = small.tile([P, 1], f32, tag="sstd")
                nc.scalar.activation(sstd, veps, AF.Sqrt, bias=eps_c)
                rstd = small.tile([P, 1], f32, tag="rstd")
                nc.vector.reciprocal(rstd, sstd)
                nmr = small.tile([P, 1], f32, tag="nmr")
                nc.vector.tensor_scalar(nmr, in0=mu, scalar1=rstd,
                                        scalar2=-1.0,
                                        op0=OP.mult, op1=OP.mult)
                # xhat = x*rstd - mu*rstd: evacuate one psum half on the DVE
                # (tensor_scalar with two per-partition scalars), the other
                # on the scalar engine.
                xo = xoutp.tile([P, E], f32, tag="xo")
                nc.vector.tensor_scalar(xo[:, 0:512], in0=psp[0],
                                        scalar1=rstd, scalar2=nmr,
                                        op0=OP.mult, op1=OP.add)
                nc.scalar.activation(xo[:, 512:1024], psp[1],
                                     AF.Identity, bias=nmr, scale=rstd)
                if apply_affine:
                    nc.gpsimd.tensor_mul(xo, xo, gamma_bc)
                    nc.gpsimd.tensor_add(xo, xo, beta_bc)
                nc.sync.dma_start(
                    out[t0:t0 + P, :], xo)

    nc.compile()
    return nc


def kernel(emb, latent, Wq, bq, Wk, bk, Wv, bv, gamma, beta):
    from concourse.bass_utils import run_bass_kernel_spmd

    gamma = np.ascontiguousarray(gamma, dtype=np.float32)
    beta = np.ascontiguousarray(beta, dtype=np.float32)
    apply_affine = not (np.all(gamma == 1.0) and np.all(beta == 0.0))

    key = ("nc", apply_affine)
    if key not in _cache:
        _cache[key] = _build(apply_affine)
    nc = _cache[key]

    emb = np.ascontiguousarray(emb, dtype=np.float32)
    latent = np.ascontiguousarray(latent, dtype=np.float32)
    shared = {
        "Wq": np.ascontiguousarray(Wq, dtype=np.float32),
        "bq": np.ascontiguousarray(bq, dtype=np.float32),
        "Wk": np.ascontiguousarray(Wk, dtype=np.float32),
        "bk": np.ascontiguousarray(bk, dtype=np.float32),
        "Wv": np.ascontiguousarray(Wv, dtype=np.float32),
        "bv": np.ascontiguousarray(bv, dtype=np.float32),
        "gamma": gamma,
        "beta": beta,
    }
    in_maps = [
        {"emb": emb[b], "latent": latent[b], **shared} for b in range(B)
    ]
    trace = bool(int(os.environ.get("KERNEL_TRACE", "0")))
    res = run_bass_kernel_spmd(nc, in_maps, list(range(B)), trace=trace)
    LAST_RUN["exec_time_ns"] = res.exec_time_ns
    LAST_RUN["mean_exec_time_ns"] = res.mean_exec_time_ns
    LAST_RUN["profile_json"] = res.profile_json
    return np.stack([res.results[b]["out"] for b in range(B)], axis=0)
